# revision 5
# baseline (speedup 1.0000x reference)
"""Trainium2 Bass kernel for a dense transformer block (nn_Block_3564822855835).

Reference computation (fp32):
    x  = x + attention(rmsnorm(x, g1), Wq, Wk, Wv, Wo)   # causal MHA, 16 heads
    out = x + gelu(rmsnorm(x, g2) @ W1 + b1) @ W2 + b2   # exact-erf gelu

Shapes: x [2, 2048, 1024], 16 heads x 64, d_ff 4096.

Distribution (8 NeuronCores, one SPMD program):
  - Token-sharded: core c owns batch c//4, token block s=c%4 (512 tokens).
  - Each core computes q/k/v for its own tokens, then one AllGather of
    (k^T, v) across its 4-core batch group gives every core the full-batch
    k/v.  Attention, o-proj, and the MLP then run fully locally on the
    core's 512 tokens with full (replicated) weights -- no further
    collectives; the host concatenates the 8 output slices.
  - Activations live channel-major ("x^T": [channel, token]) on-chip, which
    makes every matmul contraction land on the partition axis with zero
    on-chip transposes.  RMSNorm partition-dim reductions are done with a
    ones-vector matmul; scale rows are partition-broadcast on GPSIMD.
  - Causality: every core runs the same 16 k-chunk schedule; a per-core
    input mask (ones / triangle / zeros per chunk) multiplies exp(S).
  - Matmuls use fp32r (1 cycle/row at N>=256, ~1e-4 rel err); the fc2
    matmul uses bf16 to halve W2 traffic (its contribution to the output
    is small).
"""

import numpy as np
import ml_dtypes

import concourse.bass as bass
import concourse.mybir as mybir
import concourse.tile as tile
from concourse import bacc

F32 = mybir.dt.float32
F32R = mybir.dt.float32r
BF16 = mybir.dt.bfloat16
AF = mybir.ActivationFunctionType

B, T, D = 2, 2048, 1024
H, DH = 16, 64
FF = 4096
EPS = 1e-6
P = 128
N_CORES = 8
NT = 512            # tokens per core
CC = D // P         # 8 channel chunks
NCH = T // P        # 16 k-chunks per batch
FT = FF // P        # 32 ff tiles
GROUPS = [[0, 1, 2, 3], [4, 5, 6, 7]]

KV_HALF = NT * D    # elements in one half (kT or v) of the kv bounce


def _build_nc():
    nc = bacc.Bacc("TRN2", target_bir_lowering=False, debug=False,
                   num_devices=N_CORES)

    def inp(name, shape, dt=F32):
        return nc.dram_tensor(name, shape, dt, kind="ExternalInput").ap()

    xt = inp("xt", [D, NT])                 # x slice, channel-major
    wq = inp("wq", [D, D])                  # g1-folded
    wk = inp("wk", [D, D])                  # g1-folded
    wv = inp("wv", [D, D])                  # g1-folded
    wo = inp("wo", [D, D])
    w1 = inp("w1", [D, FF])                 # g2-folded
    w2 = inp("w2", [FF, D], BF16)
    b1 = inp("b1", [FF])
    b2 = inp("b2", [D])
    maskb = inp("maskb", [NCH, P, NT], BF16)  # per-core causal masks
    onesc = inp("onesc", [P, 1])
    onesv = inp("onesv", [P, NCH])
    outT = nc.dram_tensor("outT", [D, NT], F32, kind="ExternalOutput").ap()

    with tile.TileContext(nc) as tc:
        with tc.tile_pool(name="const", bufs=1) as constp, \
             tc.tile_pool(name="actsf", bufs=1) as actsf, \
             tc.tile_pool(name="actsr", bufs=3) as actsr, \
             tc.tile_pool(name="hpool", bufs=1) as hpool, \
             tc.tile_pool(name="wstr", bufs=2) as wstr, \
             tc.tile_pool(name="wstr2", bufs=2) as wstr2, \
             tc.tile_pool(name="kvp", bufs=2) as kvp, \
             tc.tile_pool(name="expp", bufs=3) as expp, \
             tc.tile_pool(name="cpp", bufs=2) as cpp, \
             tc.tile_pool(name="smallp", bufs=1) as smallp, \
             tc.tile_pool(name="psb", bufs=3, space="PSUM") as psb, \
             tc.tile_pool(name="pss", bufs=2, space="PSUM") as pss, \
             tc.tile_pool(name="psy", bufs=2, space="PSUM") as psy, \
             tc.tile_pool(name="psr", bufs=1, space="PSUM") as psr, \
             tc.tile_pool(name="dram", bufs=1, space="DRAM") as dram:

            # ---------------- constants ----------------
            onesc_sb = constp.tile([P, 1], F32R)
            nc.sync.dma_start(onesc_sb[:], onesc[:].bitcast(F32R))
            onesv_sb = constp.tile([P, NCH], F32R)
            nc.sync.dma_start(onesv_sb[:], onesv[:].bitcast(F32R))
            b1_sb = constp.tile([P, FT], F32)
            nc.sync.dma_start(b1_sb[:], b1.rearrange("(o p) -> p o", p=P))
            b2_sb = constp.tile([P, CC], F32)
            nc.sync.dma_start(b2_sb[:], b2.rearrange("(o p) -> p o", p=P))
            mask_sb = constp.tile([P, NCH, NT], BF16)
            nc.sync.dma_start(mask_sb[:], maskb.rearrange("g p q -> p g q"))
            eps_sb = constp.tile([1, 1], F32)
            nc.vector.memset(eps_sb[:], EPS)

            xT = actsf.tile([P, CC, NT], F32, tag="bigf")
            nc.sync.dma_start(xT[:], xt.rearrange("(cc p) t -> p cc t", p=P))

            # ---------------- rmsnorm (channel-major) ----------------
            def rmsnorm(src_sb, dst_tag):
                """src [P, CC, NT] f32 -> normalized [P, CC, NT] f32r."""
                prow = psr.tile([1, NT], F32, tag="prow")
                for cc in range(CC):
                    sq = smallp.tile([P, NT], F32R, tag="sq")
                    nc.vector.tensor_mul(sq[:], src_sb[:, cc, :], src_sb[:, cc, :])
                    nc.tensor.matmul(prow[:], onesc_sb[:], sq[:],
                                     start=(cc == 0), stop=(cc == CC - 1))
                srow = smallp.tile([1, NT], F32, tag="srow")
                nc.scalar.activation(srow[:], prow[:], AF.Sqrt,
                                     bias=eps_sb[:], scale=1.0 / D)
                rrow = smallp.tile([1, NT], F32, tag="rrow")
                nc.vector.reciprocal(rrow[:], srow[:])
                bc = smallp.tile([P, NT], F32, tag="bc")
                nc.gpsimd.partition_broadcast(bc[:], rrow[:])
                dst = actsr.tile([P, CC, NT], F32R, tag=dst_tag)
                for cc in range(CC):
                    nc.vector.tensor_mul(dst[:, cc, :], src_sb[:, cc, :], bc[:])
                return dst

            xn = rmsnorm(xT, "bigr")

            # ---------------- k^T, v (own tokens) + bounce ----------------
            kv_in = dram.tile([2 * KV_HALF], F32)
            kT_dram = kv_in[0:KV_HALF].rearrange("(d t) -> d t", t=NT)
            v_dram = kv_in[KV_HALF:2 * KV_HALF].rearrange("(t d) -> t d", d=D)

            # k^T[d, t] = sum_c wk[c, d] * xn[c, t]
            for dt in range(CC):
                wt = wstr.tile([P, CC, P], F32R, tag="w81")
                nc.sync.dma_start(
                    wt[:], wk[:, dt * P:(dt + 1) * P]
                    .rearrange("(cc p) d -> p cc d", p=P).bitcast(F32R))
                pb = psb.tile([P, NT], F32, tag="pbig")
                for cc in range(CC):
                    nc.tensor.matmul(pb[:], wt[:, cc, :], xn[:, cc, :],
                                     start=(cc == 0), stop=(cc == CC - 1))
                cp = cpp.tile([P, NT], F32, tag="cp")
                nc.vector.tensor_copy(cp[:], pb[:])
                nc.sync.dma_start(kT_dram[dt * P:(dt + 1) * P, :], cp[:])

            # v[t, d] = sum_c xn[c, t] * wv[c, d]   (token-major)
            for dt in range(4):  # 4 slices of 256 along d
                wt = wstr2.tile([P, CC, 256], F32R, tag="w82")
                nc.sync.dma_start(
                    wt[:], wv[:, dt * 256:(dt + 1) * 256]
                    .rearrange("(cc p) d -> p cc d", p=P).bitcast(F32R))
                for tt in range(4):
                    pb = psb.tile([P, 256], F32, tag="pbig")
                    for cc in range(CC):
                        nc.tensor.matmul(
                            pb[:], xn[:, cc, tt * P:(tt + 1) * P], wt[:, cc, :],
                            start=(cc == 0), stop=(cc == CC - 1))
                    cp = cpp.tile([P, NT], F32, tag="cp")
                    nc.vector.tensor_copy(cp[:, 0:256], pb[:])
                    nc.sync.dma_start(
                        v_dram[tt * P:(tt + 1) * P, dt * 256:(dt + 1) * 256],
                        cp[:, 0:256])

            # ---------------- AllGather k/v across batch group ----------------
            agout = dram.tile([4, 2 * KV_HALF], F32)
            nc.gpsimd.collective_compute(
                "AllGather", mybir.AluOpType.bypass, replica_groups=GROUPS,
                ins=[kv_in[:].opt()], outs=[agout[:].opt()])

            # ---------------- q^T (own tokens) ----------------
            qT = actsr.tile([P, CC, NT], F32R, tag="bigr")
            for dt in range(CC):
                wt = wstr.tile([P, CC, P], F32R, tag="w81")
                nc.sync.dma_start(
                    wt[:], wq[:, dt * P:(dt + 1) * P]
                    .rearrange("(cc p) d -> p cc d", p=P).bitcast(F32R))
                pb = psb.tile([P, NT], F32, tag="pbig")
                for cc in range(CC):
                    nc.tensor.matmul(pb[:], wt[:, cc, :], xn[:, cc, :],
                                     start=(cc == 0), stop=(cc == CC - 1))
                nc.vector.tensor_copy(qT[:, dt, :], pb[:])

            # ---------------- attention ----------------
            yT = actsr.tile([P, CC, NT], F32R, tag="bigr")
            for hp in range(H // 2):
                kh = kvp.tile([P, NCH, P], F32R, tag="kh")
                for r in range(4):
                    k_src = (agout[r, 0:KV_HALF]
                             .rearrange("(d t) -> d t", t=NT)
                             [hp * P:(hp + 1) * P, :])
                    nc.sync.dma_start(
                        kh[:, 4 * r:4 * (r + 1), :],
                        k_src.rearrange("d (c p) -> d c p", p=P).bitcast(F32R))
                for h in (2 * hp, 2 * hp + 1):
                    lo = 64 * (h % 2)
                    va = kvp.tile([P, NCH, DH + 1], F32R, tag="va")
                    for r in range(4):
                        v_src = (agout[r, KV_HALF:2 * KV_HALF]
                                 .rearrange("(t d) -> t d", d=D)
                                 [:, h * DH:(h + 1) * DH])
                        nc.sync.dma_start(
                            va[:, 4 * r:4 * (r + 1), 0:DH],
                            v_src.rearrange("(c p) d -> p c d", p=P).bitcast(F32R))
                    nc.sync.dma_start(va[:, :, DH:DH + 1],
                                      onesv_sb[:, :, None])

                    qh = qT[lo:lo + 64, hp, :]
                    py = psy.tile([DH + 1, NT], F32, tag="py")
                    for g in range(NCH):
                        ps = pss.tile([P, NT], F32, tag="ps")
                        nc.tensor.matmul(ps[:], kh[lo:lo + 64, g, :], qh,
                                         start=True, stop=True)
                        es = expp.tile([P, NT], F32R, tag="es")
                        nc.scalar.activation(es[:], ps[:], AF.Exp, scale=0.125)
                        nc.vector.tensor_mul(es[:], es[:], mask_sb[:, g, :])
                        nc.tensor.matmul(py[:], va[:, g, :], es[:],
                                         start=(g == 0), stop=(g == NCH - 1))
                    rrow = smallp.tile([1, NT], F32, tag="arec")
                    nc.vector.reciprocal(rrow[:], py[DH:DH + 1, :])
                    bc = smallp.tile([64, NT], F32, tag="abc")
                    nc.gpsimd.partition_broadcast(bc[:], rrow[:])
                    nc.vector.tensor_mul(
                        yT[lo:lo + 64, hp, :],
                        py[0:DH, :], bc[:])

            # ---------------- o-proj + residual ----------------
            for ct in range(CC):
                wt = wstr.tile([P, CC, P], F32R, tag="w81")
                nc.sync.dma_start(
                    wt[:], wo[:, ct * P:(ct + 1) * P]
                    .rearrange("(cc p) c -> p cc c", p=P).bitcast(F32R))
                pb = psb.tile([P, NT], F32, tag="pbig")
                for cc in range(CC):
                    nc.tensor.matmul(pb[:], wt[:, cc, :], yT[:, cc, :],
                                     start=(cc == 0), stop=(cc == CC - 1))
                nc.vector.tensor_add(xT[:, ct, :], pb[:], xT[:, ct, :])

            # ---------------- rmsnorm2 + fc1 + gelu ----------------
            xn2 = rmsnorm(xT, "bigr")
            hT = hpool.tile([P, FT, NT], BF16)
            for ff in range(FT):
                wt = wstr.tile([P, CC, P], F32R, tag="w81")
                nc.sync.dma_start(
                    wt[:], w1[:, ff * P:(ff + 1) * P]
                    .rearrange("(cc p) d -> p cc d", p=P).bitcast(F32R))
                pb = psb.tile([P, NT], F32, tag="pbig")
                for cc in range(CC):
                    nc.tensor.matmul(pb[:], wt[:, cc, :], xn2[:, cc, :],
                                     start=(cc == 0), stop=(cc == CC - 1))
                nc.scalar.activation(hT[:, ff, :], pb[:], AF.Gelu,
                                     bias=b1_sb[:, ff:ff + 1])

            # ---------------- fc2 + bias + residual ----------------
            for ct in range(CC):
                pb = psb.tile([P, NT], F32, tag="pbig")
                for half in range(2):
                    wt = wstr2.tile([P, FT // 2, P], BF16, tag="w16")
                    nc.sync.dma_start(
                        wt[:], w2[half * (FF // 2):(half + 1) * (FF // 2),
                                  ct * P:(ct + 1) * P]
                        .rearrange("(fo p) c -> p fo c", p=P))
                    for fo in range(FT // 2):
                        ffc = half * (FT // 2) + fo
                        nc.tensor.matmul(pb[:], wt[:, fo, :], hT[:, ffc, :],
                                         start=(ffc == 0), stop=(ffc == FT - 1))
                oc = cpp.tile([P, NT], F32, tag="cp")
                nc.vector.tensor_add(oc[:], pb[:], xT[:, ct, :])
                nc.vector.tensor_scalar_add(oc[:], oc[:], b2_sb[:, ct:ct + 1])
                nc.sync.dma_start(outT[ct * P:(ct + 1) * P, :], oc[:])

    nc.compile()
    return nc


_CACHE = {}


def _get_compiled():
    """Build the Bass program and a jitted 8-core PJRT executable once."""
    if "fn" in _CACHE:
        return _CACHE["fn"]

    import jax
    from jax.sharding import Mesh, PartitionSpec
    from jax.experimental.shard_map import shard_map
    from concourse.bass2jax import (_bass_exec_p, install_neuronx_cc_hook,
                                    partition_id_tensor)

    nc = _build_nc()
    install_neuronx_cc_hook()

    partition_name = (nc.partition_id_tensor.name
                      if nc.partition_id_tensor else None)
    in_names, out_names, out_avals = [], [], []
    for alloc in nc.m.functions[0].allocations:
        if not isinstance(alloc, mybir.MemoryLocationSet):
            continue
        name = alloc.memorylocations[0].name
        if alloc.kind == "ExternalInput":
            if name != partition_name:
                in_names.append(name)
        elif alloc.kind == "ExternalOutput":
            out_names.append(name)
            out_avals.append(jax.core.ShapedArray(
                tuple(alloc.tensor_shape), mybir.dt.np(alloc.dtype)))
    n_params = len(in_names)
    all_names = list(in_names) + list(out_names)
    if partition_name is not None:
        all_names.append(partition_name)

    def _body(*args):
        operands = list(args)
        if partition_name is not None:
            operands.append(partition_id_tensor())
        outs = _bass_exec_p.bind(
            *operands,
            out_avals=tuple(out_avals),
            in_names=tuple(all_names),
            out_names=tuple(out_names),
            lowering_input_output_aliases=(),
            sim_require_finite=True,
            sim_require_nnan=True,
            nc=nc,
        )
        return tuple(outs)

    devices = jax.devices()[:N_CORES]
    mesh = Mesh(np.asarray(devices), ("core",))
    sharded = jax.jit(shard_map(
        _body, mesh=mesh,
        in_specs=(PartitionSpec("core"),) * (n_params + len(out_names)),
        out_specs=(PartitionSpec("core"),) * len(out_names),
        check_rep=False))

    _CACHE["fn"] = (sharded, in_names, out_names, out_avals)
    return _CACHE["fn"]


def _host_inputs(x, Wq, Wk, Wv, Wo, W1, b1, W2, b2, g1, g2):
    """Per-core input dicts (all keys identically shaped across cores)."""
    wq = (g1[:, None] * Wq).astype(np.float32)
    wk = (g1[:, None] * Wk).astype(np.float32)
    wv = (g1[:, None] * Wv).astype(np.float32)
    w1 = (g2[:, None] * W1).astype(np.float32)
    w2 = W2.astype(ml_dtypes.bfloat16)
    onesc = np.ones((P, 1), np.float32)
    onesv = np.ones((P, NCH), np.float32)

    per_core = []
    for c in range(N_CORES):
        b, s = divmod(c, 4)
        xt = np.ascontiguousarray(
            x[b, s * NT:(s + 1) * NT, :].T).astype(np.float32)
        mask = np.zeros((NCH, P, NT), np.float32)
        for g in range(NCH):
            if g < 4 * s:
                mask[g] = 1.0
            elif g < 4 * s + 4:
                d = g - 4 * s
                k_idx = np.arange(P)[:, None]
                q_idx = np.arange(NT)[None, :]
                mask[g] = (P * d + k_idx <= q_idx).astype(np.float32)
        per_core.append(dict(
            xt=xt, wq=wq, wk=wk, wv=wv, wo=Wo.astype(np.float32),
            w1=w1, w2=w2, b1=b1.astype(np.float32), b2=b2.astype(np.float32),
            maskb=mask.astype(ml_dtypes.bfloat16),
            onesc=onesc, onesv=onesv))
    return per_core


def _run(per_core):
    sharded, in_names, out_names, out_avals = _get_compiled()
    concat = [np.concatenate([np.asarray(per_core[c][n])
                              for c in range(N_CORES)], axis=0)
              for n in in_names]
    concat += [np.zeros((N_CORES * a.shape[0], *a.shape[1:]), a.dtype)
               for a in out_avals]
    outs = sharded(*concat)
    res = np.asarray(outs[out_names.index("outT")])
    return res.reshape(N_CORES, D, NT)


def kernel(**inputs):
    np_in = {k: np.asarray(v) for k, v in inputs.items()}
    per_core = _host_inputs(**np_in)
    res = _run(per_core)
    out = np.empty((B, T, D), np.float32)
    for c in range(N_CORES):
        b, s = divmod(c, 4)
        out[b, s * NT:(s + 1) * NT, :] = res[c].T
    return out


# Expose the compiled runner for benchmarking from test.py.
def _bench_handles(inputs):
    per_core = _host_inputs(**{k: np.asarray(v) for k, v in inputs.items()})
    sharded, in_names, out_names, out_avals = _get_compiled()
    concat = [np.concatenate([np.asarray(per_core[c][n])
                              for c in range(N_CORES)], axis=0)
              for n in in_names]
    concat += [np.zeros((N_CORES * a.shape[0], *a.shape[1:]), a.dtype)
               for a in out_avals]
    import jax
    dev_args = [jax.device_put(a) for a in concat]
    return sharded, dev_args


# revision 7
# speedup vs baseline: 1.1104x; 1.1104x over previous
"""Trainium2 Bass kernel for a dense transformer block (nn_Block_3564822855835).

Reference computation (fp32):
    x  = x + attention(rmsnorm(x, g1), Wq, Wk, Wv, Wo)   # causal MHA, 16 heads
    out = x + gelu(rmsnorm(x, g2) @ W1 + b1) @ W2 + b2   # exact-erf gelu

Shapes: x [2, 2048, 1024], 16 heads x 64, d_ff 4096.

Distribution (8 NeuronCores, one SPMD program):
  - Token-sharded: core c owns batch c//4, token block s=c%4 (512 tokens).
  - Each core computes q/k/v for its own tokens, then one AllGather of
    (k^T, v) across its 4-core batch group gives every core the full-batch
    k/v.  Attention, o-proj, and the MLP then run fully locally on the
    core's 512 tokens with full (replicated) weights -- no further
    collectives; the host concatenates the 8 output slices.
  - Activations live channel-major ("x^T": [channel, token]) on-chip, which
    makes every matmul contraction land on the partition axis with zero
    on-chip transposes.  RMSNorm partition-dim reductions are done with a
    ones-vector matmul; scale rows are partition-broadcast on GPSIMD.
  - Causality: every core runs the same 16 k-chunk schedule; a per-core
    input mask (ones / triangle / zeros per chunk) multiplies exp(S).
  - Matmuls use fp32r (1 cycle/row at N>=256, ~1e-4 rel err); the fc2
    matmul uses bf16 to halve W2 traffic (its contribution to the output
    is small).
"""

import numpy as np
import ml_dtypes

import concourse.bass as bass
import concourse.mybir as mybir
import concourse.tile as tile
from concourse import bacc

F32 = mybir.dt.float32
F32R = mybir.dt.float32r
BF16 = mybir.dt.bfloat16
AF = mybir.ActivationFunctionType

B, T, D = 2, 2048, 1024
H, DH = 16, 64
FF = 4096
EPS = 1e-6
P = 128
N_CORES = 8
NT = 512            # tokens per core
CC = D // P         # 8 channel chunks
NCH = T // P        # 16 k-chunks per batch
FT = FF // P        # 32 ff tiles
GROUPS = [[0, 1, 2, 3], [4, 5, 6, 7]]

KV_HALF = NT * D    # elements in one half (kT or v) of the kv bounce


def _build_nc(with_collective=True):
    nc = bacc.Bacc("TRN2", target_bir_lowering=False, debug=False,
                   num_devices=N_CORES)

    def inp(name, shape, dt=F32):
        return nc.dram_tensor(name, shape, dt, kind="ExternalInput").ap()

    xt = inp("xt", [D, NT])                 # x slice, channel-major
    wq = inp("wq", [D, D], BF16)            # g1-folded
    wk = inp("wk", [D, D], BF16)            # g1-folded
    wv = inp("wv", [D, D], BF16)            # g1-folded
    wo = inp("wo", [D, D], BF16)
    w1 = inp("w1", [D, FF], BF16)           # g2-folded
    w2 = inp("w2", [FF, D], BF16)
    b1 = inp("b1", [FF])
    b2 = inp("b2", [D])
    maskb = inp("maskb", [NCH, P, NT], BF16)  # per-core causal masks
    onesc = inp("onesc", [P, 1])
    onesv = inp("onesv", [P, NCH], BF16)
    outT = nc.dram_tensor("outT", [D, NT], F32, kind="ExternalOutput").ap()

    with tile.TileContext(nc) as tc:
        with tc.tile_pool(name="const", bufs=1) as constp, \
             tc.tile_pool(name="actsf", bufs=1) as actsf, \
             tc.tile_pool(name="actsr", bufs=3) as actsr, \
             tc.tile_pool(name="hpool", bufs=1) as hpool, \
             tc.tile_pool(name="wstr", bufs=2) as wstr, \
             tc.tile_pool(name="wstr2", bufs=2) as wstr2, \
             tc.tile_pool(name="kvp", bufs=2) as kvp, \
             tc.tile_pool(name="expp", bufs=3) as expp, \
             tc.tile_pool(name="cpp", bufs=2) as cpp, \
             tc.tile_pool(name="smallp", bufs=1) as smallp, \
             tc.tile_pool(name="psb", bufs=3, space="PSUM") as psb, \
             tc.tile_pool(name="pss", bufs=2, space="PSUM") as pss, \
             tc.tile_pool(name="psy", bufs=2, space="PSUM") as psy, \
             tc.tile_pool(name="psr", bufs=1, space="PSUM") as psr, \
             tc.tile_pool(name="dram", bufs=1, space="DRAM") as dram:

            # ---------------- constants ----------------
            onesc_sb = constp.tile([P, 1], F32R)
            nc.sync.dma_start(onesc_sb[:], onesc[:].bitcast(F32R))
            onesv_sb = constp.tile([P, NCH], BF16)
            nc.sync.dma_start(onesv_sb[:], onesv[:])
            b1_sb = constp.tile([P, FT], F32)
            nc.sync.dma_start(b1_sb[:], b1.rearrange("(o p) -> p o", p=P))
            b2_sb = constp.tile([P, CC], F32)
            nc.sync.dma_start(b2_sb[:], b2.rearrange("(o p) -> p o", p=P))
            mask_sb = constp.tile([P, NCH, NT], BF16)
            nc.sync.dma_start(mask_sb[:], maskb.rearrange("g p q -> p g q"))
            eps_sb = constp.tile([1, 1], F32)
            nc.vector.memset(eps_sb[:], EPS)

            xT = actsf.tile([P, CC, NT], F32, tag="bigf")
            nc.sync.dma_start(xT[:], xt.rearrange("(cc p) t -> p cc t", p=P))

            # ---------------- rmsnorm (channel-major) ----------------
            def rmsnorm(src_sb, dst_tag):
                """src [P, CC, NT] f32 -> normalized [P, CC, NT] f32r."""
                prow = psr.tile([1, NT], F32, tag="prow")
                for cc in range(CC):
                    sq = smallp.tile([P, NT], F32R, tag="sq")
                    nc.vector.tensor_mul(sq[:], src_sb[:, cc, :], src_sb[:, cc, :])
                    nc.tensor.matmul(prow[:], onesc_sb[:], sq[:],
                                     start=(cc == 0), stop=(cc == CC - 1))
                srow = smallp.tile([1, NT], F32, tag="srow")
                nc.scalar.activation(srow[:], prow[:], AF.Sqrt,
                                     bias=eps_sb[:], scale=1.0 / D)
                rrow = smallp.tile([1, NT], F32, tag="rrow")
                nc.vector.reciprocal(rrow[:], srow[:])
                bc = smallp.tile([P, NT], F32, tag="bc")
                nc.gpsimd.partition_broadcast(bc[:], rrow[:])
                dst = actsr.tile([P, CC, NT], BF16, tag=dst_tag)
                for cc in range(CC):
                    nc.vector.tensor_mul(dst[:, cc, :], src_sb[:, cc, :], bc[:])
                return dst

            xn = rmsnorm(xT, "bigr")

            # ---------------- k^T, v (own tokens) + bounce ----------------
            kv_in = dram.tile([2 * KV_HALF], BF16)
            kT_dram = kv_in[0:KV_HALF].rearrange("(d t) -> d t", t=NT)
            v_dram = kv_in[KV_HALF:2 * KV_HALF].rearrange("(t d) -> t d", d=D)

            # k^T[d, t] = sum_c wk[c, d] * xn[c, t]
            for dt in range(CC):
                wt = wstr.tile([P, CC, P], BF16, tag="w81")
                nc.sync.dma_start(
                    wt[:], wk[:, dt * P:(dt + 1) * P]
                    .rearrange("(cc p) d -> p cc d", p=P))
                pb = psb.tile([P, NT], F32, tag="pbig")
                for cc in range(CC):
                    nc.tensor.matmul(pb[:], wt[:, cc, :], xn[:, cc, :],
                                     start=(cc == 0), stop=(cc == CC - 1))
                cp = cpp.tile([P, NT], BF16, tag="cpb")
                nc.vector.tensor_copy(cp[:], pb[:])
                nc.sync.dma_start(kT_dram[dt * P:(dt + 1) * P, :], cp[:])

            # v[t, d] = sum_c xn[c, t] * wv[c, d]   (token-major)
            for dt in range(2):  # 2 slices of 512 along d
                wt = wstr2.tile([P, CC, NT], BF16, tag="w82")
                nc.sync.dma_start(
                    wt[:], wv[:, dt * NT:(dt + 1) * NT]
                    .rearrange("(cc p) d -> p cc d", p=P))
                for tt in range(4):
                    pb = psb.tile([P, NT], F32, tag="pbig")
                    for cc in range(CC):
                        nc.tensor.matmul(
                            pb[:], xn[:, cc, tt * P:(tt + 1) * P], wt[:, cc, :],
                            start=(cc == 0), stop=(cc == CC - 1))
                    cp = cpp.tile([P, NT], BF16, tag="cpb")
                    nc.vector.tensor_copy(cp[:], pb[:])
                    nc.sync.dma_start(
                        v_dram[tt * P:(tt + 1) * P, dt * NT:(dt + 1) * NT],
                        cp[:])

            # ---------------- AllGather k/v across batch group ----------------
            agout = dram.tile([4, 2 * KV_HALF], BF16)
            if with_collective:
                nc.gpsimd.collective_compute(
                    "AllGather", mybir.AluOpType.bypass, replica_groups=GROUPS,
                    ins=[kv_in[:].opt()], outs=[agout[:].opt()])
            else:
                nc.sync.dma_start(agout[0, :], kv_in[:])

            # ---------------- q^T (own tokens) ----------------
            qT = actsr.tile([P, CC, NT], BF16, tag="bigr")
            for dt in range(CC):
                wt = wstr.tile([P, CC, P], BF16, tag="w81")
                nc.sync.dma_start(
                    wt[:], wq[:, dt * P:(dt + 1) * P]
                    .rearrange("(cc p) d -> p cc d", p=P))
                pb = psb.tile([P, NT], F32, tag="pbig")
                for cc in range(CC):
                    nc.tensor.matmul(pb[:], wt[:, cc, :], xn[:, cc, :],
                                     start=(cc == 0), stop=(cc == CC - 1))
                nc.vector.tensor_copy(qT[:, dt, :], pb[:])

            # ---------------- attention ----------------
            yT = actsr.tile([P, CC, NT], BF16, tag="bigr")
            for hp in range(H // 2):
                kh = kvp.tile([P, NCH, P], BF16, tag="kh")
                for r in range(4):
                    k_src = (agout[r, 0:KV_HALF]
                             .rearrange("(d t) -> d t", t=NT)
                             [hp * P:(hp + 1) * P, :])
                    nc.sync.dma_start(
                        kh[:, 4 * r:4 * (r + 1), :],
                        k_src.rearrange("d (c p) -> d c p", p=P))
                for h in (2 * hp, 2 * hp + 1):
                    lo = 64 * (h % 2)
                    va = kvp.tile([P, NCH, DH + 1], BF16, tag="va")
                    for r in range(4):
                        v_src = (agout[r, KV_HALF:2 * KV_HALF]
                                 .rearrange("(t d) -> t d", d=D)
                                 [:, h * DH:(h + 1) * DH])
                        nc.sync.dma_start(
                            va[:, 4 * r:4 * (r + 1), 0:DH],
                            v_src.rearrange("(c p) d -> p c d", p=P))
                    nc.sync.dma_start(va[:, :, DH:DH + 1],
                                      onesv_sb[:, :, None])

                    qh = qT[lo:lo + 64, hp, :]
                    py = psy.tile([DH + 1, NT], F32, tag="py")
                    for g in range(NCH):
                        ps = pss.tile([P, NT], F32, tag="ps")
                        nc.tensor.matmul(ps[:], kh[lo:lo + 64, g, :], qh,
                                         start=True, stop=True)
                        es = expp.tile([P, NT], BF16, tag="es")
                        nc.scalar.activation(es[:], ps[:], AF.Exp, scale=0.125)
                        nc.vector.tensor_mul(es[:], es[:], mask_sb[:, g, :])
                        nc.tensor.matmul(py[:], va[:, g, :], es[:],
                                         start=(g == 0), stop=(g == NCH - 1))
                    rrow = smallp.tile([1, NT], F32, tag="arec")
                    nc.vector.reciprocal(rrow[:], py[DH:DH + 1, :])
                    bc = smallp.tile([64, NT], F32, tag="abc")
                    nc.gpsimd.partition_broadcast(bc[:], rrow[:])
                    nc.vector.tensor_mul(
                        yT[lo:lo + 64, hp, :],
                        py[0:DH, :], bc[:])

            # ---------------- o-proj + residual ----------------
            for ct in range(CC):
                wt = wstr.tile([P, CC, P], BF16, tag="w81")
                nc.sync.dma_start(
                    wt[:], wo[:, ct * P:(ct + 1) * P]
                    .rearrange("(cc p) c -> p cc c", p=P))
                pb = psb.tile([P, NT], F32, tag="pbig")
                for cc in range(CC):
                    nc.tensor.matmul(pb[:], wt[:, cc, :], yT[:, cc, :],
                                     start=(cc == 0), stop=(cc == CC - 1))
                nc.vector.tensor_add(xT[:, ct, :], pb[:], xT[:, ct, :])

            # ---------------- rmsnorm2 + fc1 + gelu ----------------
            xn2 = rmsnorm(xT, "bigr")
            hT = hpool.tile([P, FT, NT], BF16)
            for ff in range(FT):
                wt = wstr.tile([P, CC, P], BF16, tag="w81")
                nc.sync.dma_start(
                    wt[:], w1[:, ff * P:(ff + 1) * P]
                    .rearrange("(cc p) d -> p cc d", p=P))
                pb = psb.tile([P, NT], F32, tag="pbig")
                for cc in range(CC):
                    nc.tensor.matmul(pb[:], wt[:, cc, :], xn2[:, cc, :],
                                     start=(cc == 0), stop=(cc == CC - 1))
                nc.scalar.activation(hT[:, ff, :], pb[:], AF.Gelu,
                                     bias=b1_sb[:, ff:ff + 1])

            # ---------------- fc2 + bias + residual ----------------
            for ct in range(CC):
                pb = psb.tile([P, NT], F32, tag="pbig")
                for half in range(2):
                    wt = wstr2.tile([P, FT // 2, P], BF16, tag="w16")
                    nc.sync.dma_start(
                        wt[:], w2[half * (FF // 2):(half + 1) * (FF // 2),
                                  ct * P:(ct + 1) * P]
                        .rearrange("(fo p) c -> p fo c", p=P))
                    for fo in range(FT // 2):
                        ffc = half * (FT // 2) + fo
                        nc.tensor.matmul(pb[:], wt[:, fo, :], hT[:, ffc, :],
                                         start=(ffc == 0), stop=(ffc == FT - 1))
                oc = cpp.tile([P, NT], F32, tag="cp")
                nc.vector.tensor_add(oc[:], pb[:], xT[:, ct, :])
                nc.vector.tensor_scalar_add(oc[:], oc[:], b2_sb[:, ct:ct + 1])
                nc.sync.dma_start(outT[ct * P:(ct + 1) * P, :], oc[:])

    nc.compile()
    return nc


_CACHE = {}


def _get_compiled():
    """Build the Bass program and a jitted 8-core PJRT executable once."""
    if "fn" in _CACHE:
        return _CACHE["fn"]

    import jax
    from jax.sharding import Mesh, PartitionSpec
    from jax.experimental.shard_map import shard_map
    from concourse.bass2jax import (_bass_exec_p, install_neuronx_cc_hook,
                                    partition_id_tensor)

    nc = _build_nc()
    install_neuronx_cc_hook()

    partition_name = (nc.partition_id_tensor.name
                      if nc.partition_id_tensor else None)
    in_names, out_names, out_avals = [], [], []
    for alloc in nc.m.functions[0].allocations:
        if not isinstance(alloc, mybir.MemoryLocationSet):
            continue
        name = alloc.memorylocations[0].name
        if alloc.kind == "ExternalInput":
            if name != partition_name:
                in_names.append(name)
        elif alloc.kind == "ExternalOutput":
            out_names.append(name)
            out_avals.append(jax.core.ShapedArray(
                tuple(alloc.tensor_shape), mybir.dt.np(alloc.dtype)))
    n_params = len(in_names)
    all_names = list(in_names) + list(out_names)
    if partition_name is not None:
        all_names.append(partition_name)

    def _body(*args):
        operands = list(args)
        if partition_name is not None:
            operands.append(partition_id_tensor())
        outs = _bass_exec_p.bind(
            *operands,
            out_avals=tuple(out_avals),
            in_names=tuple(all_names),
            out_names=tuple(out_names),
            lowering_input_output_aliases=(),
            sim_require_finite=True,
            sim_require_nnan=True,
            nc=nc,
        )
        return tuple(outs)

    devices = jax.devices()[:N_CORES]
    mesh = Mesh(np.asarray(devices), ("core",))
    sharded = jax.jit(shard_map(
        _body, mesh=mesh,
        in_specs=(PartitionSpec("core"),) * (n_params + len(out_names)),
        out_specs=(PartitionSpec("core"),) * len(out_names),
        check_rep=False))

    _CACHE["fn"] = (sharded, in_names, out_names, out_avals)
    return _CACHE["fn"]


def _host_inputs(x, Wq, Wk, Wv, Wo, W1, b1, W2, b2, g1, g2):
    """Per-core input dicts (all keys identically shaped across cores)."""
    wq = (g1[:, None] * Wq).astype(ml_dtypes.bfloat16)
    wk = (g1[:, None] * Wk).astype(ml_dtypes.bfloat16)
    wv = (g1[:, None] * Wv).astype(ml_dtypes.bfloat16)
    w1 = (g2[:, None] * W1).astype(ml_dtypes.bfloat16)
    w2 = W2.astype(ml_dtypes.bfloat16)
    onesc = np.ones((P, 1), np.float32)
    onesv = np.ones((P, NCH), ml_dtypes.bfloat16)

    per_core = []
    for c in range(N_CORES):
        b, s = divmod(c, 4)
        xt = np.ascontiguousarray(
            x[b, s * NT:(s + 1) * NT, :].T).astype(np.float32)
        mask = np.zeros((NCH, P, NT), np.float32)
        for g in range(NCH):
            if g < 4 * s:
                mask[g] = 1.0
            elif g < 4 * s + 4:
                d = g - 4 * s
                k_idx = np.arange(P)[:, None]
                q_idx = np.arange(NT)[None, :]
                mask[g] = (P * d + k_idx <= q_idx).astype(np.float32)
        per_core.append(dict(
            xt=xt, wq=wq, wk=wk, wv=wv, wo=Wo.astype(ml_dtypes.bfloat16),
            w1=w1, w2=w2, b1=b1.astype(np.float32), b2=b2.astype(np.float32),
            maskb=mask.astype(ml_dtypes.bfloat16),
            onesc=onesc, onesv=onesv))
    return per_core


def _run(per_core):
    sharded, in_names, out_names, out_avals = _get_compiled()
    concat = [np.concatenate([np.asarray(per_core[c][n])
                              for c in range(N_CORES)], axis=0)
              for n in in_names]
    concat += [np.zeros((N_CORES * a.shape[0], *a.shape[1:]), a.dtype)
               for a in out_avals]
    outs = sharded(*concat)
    res = np.asarray(outs[out_names.index("outT")])
    return res.reshape(N_CORES, D, NT)


def kernel(**inputs):
    np_in = {k: np.asarray(v) for k, v in inputs.items()}
    per_core = _host_inputs(**np_in)
    res = _run(per_core)
    out = np.empty((B, T, D), np.float32)
    for c in range(N_CORES):
        b, s = divmod(c, 4)
        out[b, s * NT:(s + 1) * NT, :] = res[c].T
    return out


# Expose the compiled runner for benchmarking from test.py.
def _bench_handles(inputs):
    per_core = _host_inputs(**{k: np.asarray(v) for k, v in inputs.items()})
    sharded, in_names, out_names, out_avals = _get_compiled()
    concat = [np.concatenate([np.asarray(per_core[c][n])
                              for c in range(N_CORES)], axis=0)
              for n in in_names]
    concat += [np.zeros((N_CORES * a.shape[0], *a.shape[1:]), a.dtype)
               for a in out_avals]
    import jax
    dev_args = [jax.device_put(a) for a in concat]
    return sharded, dev_args


# revision 8
# speedup vs baseline: 1.1383x; 1.0251x over previous
"""Trainium2 Bass kernel for a dense transformer block (nn_Block_3564822855835).

Reference computation (fp32):
    x  = x + attention(rmsnorm(x, g1), Wq, Wk, Wv, Wo)   # causal MHA, 16 heads
    out = x + gelu(rmsnorm(x, g2) @ W1 + b1) @ W2 + b2   # exact-erf gelu

Shapes: x [2, 2048, 1024], 16 heads x 64, d_ff 4096.

Distribution (8 NeuronCores, one SPMD program, DeepSpeed-Ulysses style):
  - Token-sharded outside attention: core c owns batch c//4, token block
    c%4 (512 tokens): rmsnorm + q/k/v projections + o-proj + MLP all run
    on the core's own 512 tokens with full (replicated) weights.
  - Head-sharded inside attention: an 8-core AllToAll redistributes
    (q^T, k^T, v) from token-shards to head-shards; each core then runs
    causal attention for its 2 heads over the full 4096 tokens, and a
    second AllToAll routes y^T back to token shards.  Head sharding makes
    the causal structure identical on every core, so future k-chunks are
    statically skipped and the diagonal masks sit at static slots --
    full causal savings with a single SPMD program.
  - Activations are channel-major ("x^T": [channel, token]) on-chip so all
    matmul contractions land on the partition axis with zero on-chip
    transposes.  Partition-dim reductions (rmsnorm sum, softmax sum) use
    a ones-column fused into the matmuls; scale rows are partition-
    broadcast on GPSIMD.
  - Matmul operands are bf16 (fp32 PSUM accumulation); the residual
    stream and all softmax/norm statistics stay fp32.
"""

import numpy as np
import ml_dtypes

import concourse.bass as bass
import concourse.mybir as mybir
import concourse.tile as tile
from concourse import bacc

F32 = mybir.dt.float32
F32R = mybir.dt.float32r
BF16 = mybir.dt.bfloat16
AF = mybir.ActivationFunctionType

B, T, D = 2, 2048, 1024
H, DH = 16, 64
FF = 4096
EPS = 1e-6
P = 128
N_CORES = 8
NT = 512            # tokens per core
CC = D // P         # 8 channel chunks
NCH = T // P        # 16 k-chunks per batch
FT = FF // P        # 32 ff tiles
PNT = P * NT        # elements in one [128, 512] plane

ALL8 = [[0, 1, 2, 3, 4, 5, 6, 7]]


def _build_nc(with_collective=True):
    nc = bacc.Bacc("TRN2", target_bir_lowering=False, debug=False,
                   num_devices=N_CORES)

    def inp(name, shape, dt=F32):
        return nc.dram_tensor(name, shape, dt, kind="ExternalInput").ap()

    xt = inp("xt", [D, NT])                 # x slice, channel-major
    wq = inp("wq", [D, D], BF16)            # g1-folded
    wk = inp("wk", [D, D], BF16)            # g1-folded
    wv = inp("wv", [D, D], BF16)            # g1-folded
    wo = inp("wo", [D, D], BF16)
    w1 = inp("w1", [D, FF], BF16)           # g2-folded
    w2 = inp("w2", [FF, D], BF16)
    b1 = inp("b1", [FF])
    b2 = inp("b2", [D])
    dmask = inp("dmask", [4, P, NT], BF16)  # diagonal causal masks (global)
    onesc = inp("onesc", [P, 1])
    onesv = inp("onesv", [P, NCH], BF16)
    outT = nc.dram_tensor("outT", [D, NT], F32, kind="ExternalOutput").ap()

    with tile.TileContext(nc) as tc:
        with tc.tile_pool(name="const", bufs=1) as constp, \
             tc.tile_pool(name="actsf", bufs=1) as actsf, \
             tc.tile_pool(name="actsr", bufs=3) as actsr, \
             tc.tile_pool(name="hpool", bufs=1) as hpool, \
             tc.tile_pool(name="wstr", bufs=4) as wstr, \
             tc.tile_pool(name="wstr2", bufs=2) as wstr2, \
             tc.tile_pool(name="kvp", bufs=2) as kvp, \
             tc.tile_pool(name="expp", bufs=5) as expp, \
             tc.tile_pool(name="cpp", bufs=3) as cpp, \
             tc.tile_pool(name="smallp", bufs=2) as smallp, \
             tc.tile_pool(name="psb", bufs=3, space="PSUM") as psb, \
             tc.tile_pool(name="pss", bufs=2, space="PSUM") as pss, \
             tc.tile_pool(name="psy", bufs=2, space="PSUM") as psy, \
             tc.tile_pool(name="psr", bufs=1, space="PSUM") as psr, \
             tc.tile_pool(name="dram", bufs=1, space="DRAM") as dram:

            # ---------------- constants ----------------
            onesc_sb = constp.tile([P, 1], F32R)
            nc.sync.dma_start(onesc_sb[:], onesc[:].bitcast(F32R))
            onesv_sb = constp.tile([P, NCH], BF16)
            nc.sync.dma_start(onesv_sb[:], onesv[:])
            b1_sb = constp.tile([P, FT], F32)
            nc.sync.dma_start(b1_sb[:], b1.rearrange("(o p) -> p o", p=P))
            b2_sb = constp.tile([P, CC], F32)
            nc.sync.dma_start(b2_sb[:], b2.rearrange("(o p) -> p o", p=P))
            mask_sb = constp.tile([P, 4, NT], BF16)
            nc.sync.dma_start(mask_sb[:], dmask.rearrange("g p q -> p g q"))
            eps_sb = constp.tile([1, 1], F32)
            nc.vector.memset(eps_sb[:], EPS)

            xT = actsf.tile([P, CC, NT], F32, tag="bigf")
            nc.sync.dma_start(xT[:], xt.rearrange("(cc p) t -> p cc t", p=P))

            # ---------------- rmsnorm (channel-major) ----------------
            def rmsnorm(src_sb):
                """src [P, CC, NT] f32 -> normalized [P, CC, NT] bf16."""
                prow = psr.tile([1, NT], F32, tag="prow")
                for cc in range(CC):
                    sq = smallp.tile([P, NT], F32R, tag="sq")
                    nc.vector.tensor_mul(sq[:], src_sb[:, cc, :], src_sb[:, cc, :])
                    nc.tensor.matmul(prow[:], onesc_sb[:], sq[:],
                                     start=(cc == 0), stop=(cc == CC - 1))
                srow = smallp.tile([1, NT], F32, tag="srow")
                nc.scalar.activation(srow[:], prow[:], AF.Sqrt,
                                     bias=eps_sb[:], scale=1.0 / D)
                rrow = smallp.tile([1, NT], F32, tag="rrow")
                nc.vector.reciprocal(rrow[:], srow[:])
                bc = smallp.tile([P, NT], F32, tag="bc")
                nc.gpsimd.partition_broadcast(bc[:], rrow[:])
                dst = actsr.tile([P, CC, NT], BF16, tag="bigr")
                for cc in range(CC):
                    nc.vector.tensor_mul(dst[:, cc, :], src_sb[:, cc, :], bc[:])
                return dst

            xn = rmsnorm(xT)

            # ------------- q/k/v projections -> AllToAll bounce -------------
            # shard j (head pair j): q^T [128,512] | k^T [128,512] | v [512,128]
            a2a_in = dram.tile([N_CORES, 3 * PNT], BF16)
            a2a_out = dram.tile([N_CORES, 3 * PNT], BF16)

            def proj_qk(w, region):
                for dt in range(CC):
                    wt = wstr.tile([P, CC, P], BF16, tag="w81")
                    nc.sync.dma_start(
                        wt[:], w[:, dt * P:(dt + 1) * P]
                        .rearrange("(cc p) d -> p cc d", p=P))
                    pb = psb.tile([P, NT], F32, tag="pbig")
                    for cc in range(CC):
                        nc.tensor.matmul(pb[:], wt[:, cc, :], xn[:, cc, :],
                                         start=(cc == 0), stop=(cc == CC - 1))
                    cp = cpp.tile([P, NT], BF16, tag="cpb")
                    nc.vector.tensor_copy(cp[:], pb[:])
                    dst = (a2a_in[dt, region * PNT:(region + 1) * PNT]
                           .rearrange("(p t) -> p t", t=NT))
                    nc.sync.dma_start(dst, cp[:])

            proj_qk(wq, 0)
            proj_qk(wk, 1)

            # v token-major: v[t, d] = sum_c xn[c, t] wv[c, d]
            for dt in range(2):
                wt = wstr2.tile([P, CC, NT], BF16, tag="w82")
                nc.sync.dma_start(
                    wt[:], wv[:, dt * NT:(dt + 1) * NT]
                    .rearrange("(cc p) d -> p cc d", p=P))
                for tt in range(4):
                    pb = psb.tile([P, NT], F32, tag="pbig")
                    for cc in range(CC):
                        nc.tensor.matmul(
                            pb[:], xn[:, cc, tt * P:(tt + 1) * P], wt[:, cc, :],
                            start=(cc == 0), stop=(cc == CC - 1))
                    cp = cpp.tile([P, NT], BF16, tag="cpb")
                    nc.vector.tensor_copy(cp[:], pb[:])
                    for u in range(4):
                        j = 4 * dt + u
                        dst = (a2a_in[j, 2 * PNT:3 * PNT]
                               .rearrange("(t d) -> t d", d=P)
                               [tt * P:(tt + 1) * P, :])
                        nc.sync.dma_start(dst, cp[:, u * P:(u + 1) * P])

            if with_collective:
                nc.gpsimd.collective_compute(
                    "AllToAll", mybir.AluOpType.bypass, replica_groups=ALL8,
                    ins=[a2a_in[:].opt()], outs=[a2a_out[:].opt()])
            else:
                nc.sync.dma_start(a2a_out[:], a2a_in[:])

            # ------------- attention (my 2 heads, all tokens) -------------
            yTm = actsr.tile([P, CC, NT], BF16, tag="bigr")
            for b in range(B):
                kh = kvp.tile([P, NCH, P], BF16, tag="kh")
                qh = kvp.tile([P, 4, NT], BF16, tag="qh")
                for rr in range(4):
                    r = 4 * b + rr
                    k_v = a2a_out[r, PNT:2 * PNT].rearrange("(p t) -> p t", t=NT)
                    nc.sync.dma_start(
                        kh[:, 4 * rr:4 * rr + 4, :],
                        k_v.rearrange("p (c q) -> p c q", q=P))
                    q_v = a2a_out[r, 0:PNT].rearrange("(p t) -> p t", t=NT)
                    nc.sync.dma_start(qh[:, rr, :], q_v)
                for hh in range(2):
                    lo = 64 * hh
                    va = kvp.tile([P, NCH, DH + 1], BF16, tag="va")
                    for rr in range(4):
                        r = 4 * b + rr
                        v_v = (a2a_out[r, 2 * PNT:3 * PNT]
                               .rearrange("(t d) -> t d", d=P)
                               [:, lo:lo + DH])
                        nc.sync.dma_start(
                            va[:, 4 * rr:4 * rr + 4, 0:DH],
                            v_v.rearrange("(c p) d -> p c d", p=P))
                    nc.sync.dma_start(va[:, :, DH:DH + 1], onesv_sb[:, :, None])

                    for qb in range(4):
                        py = psy.tile([DH + 1, NT], F32, tag="py")
                        nch = 4 * qb + 4
                        for g in range(nch):
                            ps = pss.tile([P, NT], F32, tag="ps")
                            nc.tensor.matmul(ps[:], kh[lo:lo + 64, g, :],
                                             qh[lo:lo + 64, qb, :],
                                             start=True, stop=True)
                            es = expp.tile([P, NT], BF16, tag="es")
                            nc.scalar.activation(es[:], ps[:], AF.Exp,
                                                 scale=0.125)
                            if g >= 4 * qb:
                                nc.vector.tensor_mul(es[:], es[:],
                                                     mask_sb[:, g - 4 * qb, :])
                            nc.tensor.matmul(py[:], va[:, g, :], es[:],
                                             start=(g == 0), stop=(g == nch - 1))
                        rrow = smallp.tile([1, NT], F32, tag="arec")
                        nc.vector.reciprocal(rrow[:], py[DH:DH + 1, :])
                        bcy = smallp.tile([64, NT], F32, tag="abc")
                        nc.gpsimd.partition_broadcast(bcy[:], rrow[:])
                        nc.vector.tensor_mul(
                            yTm[lo:lo + 64, 4 * b + qb, :],
                            py[0:DH, :], bcy[:])

            # ------------- AllToAll y back to token shards -------------
            a2a2_in = dram.tile([N_CORES, PNT], BF16)
            a2a2_out = dram.tile([N_CORES, PNT], BF16)
            nc.sync.dma_start(
                a2a2_in[:, :].rearrange("r (p t) -> p r t", t=NT), yTm[:])
            if with_collective:
                nc.gpsimd.collective_compute(
                    "AllToAll", mybir.AluOpType.bypass, replica_groups=ALL8,
                    ins=[a2a2_in[:].opt()], outs=[a2a2_out[:].opt()])
            else:
                nc.sync.dma_start(a2a2_out[:], a2a2_in[:])
            yTf = actsr.tile([P, CC, NT], BF16, tag="bigr")
            nc.sync.dma_start(
                yTf[:], a2a2_out[:, :].rearrange("r (p t) -> p r t", t=NT))

            # ------------- o-proj + residual (in place into xT) -------------
            for ct in range(CC):
                wt = wstr.tile([P, CC, P], BF16, tag="w81")
                nc.sync.dma_start(
                    wt[:], wo[:, ct * P:(ct + 1) * P]
                    .rearrange("(cc p) c -> p cc c", p=P))
                pb = psb.tile([P, NT], F32, tag="pbig")
                for cc in range(CC):
                    nc.tensor.matmul(pb[:], wt[:, cc, :], yTf[:, cc, :],
                                     start=(cc == 0), stop=(cc == CC - 1))
                nc.vector.tensor_add(xT[:, ct, :], pb[:], xT[:, ct, :])

            # ---------------- rmsnorm2 + fc1 + gelu ----------------
            xn2 = rmsnorm(xT)
            hT = hpool.tile([P, FT, NT], BF16)
            for ff in range(FT):
                wt = wstr.tile([P, CC, P], BF16, tag="w81")
                nc.sync.dma_start(
                    wt[:], w1[:, ff * P:(ff + 1) * P]
                    .rearrange("(cc p) d -> p cc d", p=P))
                pb = psb.tile([P, NT], F32, tag="pbig")
                for cc in range(CC):
                    nc.tensor.matmul(pb[:], wt[:, cc, :], xn2[:, cc, :],
                                     start=(cc == 0), stop=(cc == CC - 1))
                nc.scalar.activation(hT[:, ff, :], pb[:], AF.Gelu,
                                     bias=b1_sb[:, ff:ff + 1])

            # ---------------- fc2 + bias + residual ----------------
            for ct in range(CC):
                pb = psb.tile([P, NT], F32, tag="pbig")
                for half in range(2):
                    wt = wstr2.tile([P, FT // 2, P], BF16, tag="w16")
                    nc.sync.dma_start(
                        wt[:], w2[half * (FF // 2):(half + 1) * (FF // 2),
                                  ct * P:(ct + 1) * P]
                        .rearrange("(fo p) c -> p fo c", p=P))
                    for fo in range(FT // 2):
                        ffc = half * (FT // 2) + fo
                        nc.tensor.matmul(pb[:], wt[:, fo, :], hT[:, ffc, :],
                                         start=(ffc == 0), stop=(ffc == FT - 1))
                oc = cpp.tile([P, NT], F32, tag="cp")
                nc.vector.tensor_add(oc[:], pb[:], xT[:, ct, :])
                nc.vector.tensor_scalar_add(oc[:], oc[:], b2_sb[:, ct:ct + 1])
                nc.sync.dma_start(outT[ct * P:(ct + 1) * P, :], oc[:])

    nc.compile()
    return nc


_CACHE = {}


def _get_compiled():
    """Build the Bass program and a jitted 8-core PJRT executable once."""
    if "fn" in _CACHE:
        return _CACHE["fn"]

    import jax
    from jax.sharding import Mesh, PartitionSpec
    from jax.experimental.shard_map import shard_map
    from concourse.bass2jax import (_bass_exec_p, install_neuronx_cc_hook,
                                    partition_id_tensor)

    nc = _build_nc()
    install_neuronx_cc_hook()

    partition_name = (nc.partition_id_tensor.name
                      if nc.partition_id_tensor else None)
    in_names, out_names, out_avals = [], [], []
    for alloc in nc.m.functions[0].allocations:
        if not isinstance(alloc, mybir.MemoryLocationSet):
            continue
        name = alloc.memorylocations[0].name
        if alloc.kind == "ExternalInput":
            if name != partition_name:
                in_names.append(name)
        elif alloc.kind == "ExternalOutput":
            out_names.append(name)
            out_avals.append(jax.core.ShapedArray(
                tuple(alloc.tensor_shape), mybir.dt.np(alloc.dtype)))
    n_params = len(in_names)
    all_names = list(in_names) + list(out_names)
    if partition_name is not None:
        all_names.append(partition_name)

    def _body(*args):
        operands = list(args)
        if partition_name is not None:
            operands.append(partition_id_tensor())
        outs = _bass_exec_p.bind(
            *operands,
            out_avals=tuple(out_avals),
            in_names=tuple(all_names),
            out_names=tuple(out_names),
            lowering_input_output_aliases=(),
            sim_require_finite=True,
            sim_require_nnan=True,
            nc=nc,
        )
        return tuple(outs)

    devices = jax.devices()[:N_CORES]
    mesh = Mesh(np.asarray(devices), ("core",))
    sharded = jax.jit(shard_map(
        _body, mesh=mesh,
        in_specs=(PartitionSpec("core"),) * (n_params + len(out_names)),
        out_specs=(PartitionSpec("core"),) * len(out_names),
        check_rep=False))

    _CACHE["fn"] = (sharded, in_names, out_names, out_avals)
    return _CACHE["fn"]


def _host_inputs(x, Wq, Wk, Wv, Wo, W1, b1, W2, b2, g1, g2):
    """Per-core input dicts (all keys identically shaped across cores)."""
    bf = ml_dtypes.bfloat16
    wq = (g1[:, None] * Wq).astype(bf)
    wk = (g1[:, None] * Wk).astype(bf)
    wv = (g1[:, None] * Wv).astype(bf)
    w1 = (g2[:, None] * W1).astype(bf)
    w2 = W2.astype(bf)
    wo = Wo.astype(bf)
    onesc = np.ones((P, 1), np.float32)
    onesv = np.ones((P, NCH), bf)
    k_idx = np.arange(P)[:, None]
    q_idx = np.arange(NT)[None, :]
    dmask = np.stack([(P * d + k_idx <= q_idx) for d in range(4)]).astype(bf)

    per_core = []
    for c in range(N_CORES):
        b, s = divmod(c, 4)
        xt = np.ascontiguousarray(
            x[b, s * NT:(s + 1) * NT, :].T).astype(np.float32)
        per_core.append(dict(
            xt=xt, wq=wq, wk=wk, wv=wv, wo=wo,
            w1=w1, w2=w2, b1=b1.astype(np.float32), b2=b2.astype(np.float32),
            dmask=dmask, onesc=onesc, onesv=onesv))
    return per_core


def _concat_inputs(per_core, in_names, out_avals):
    concat = [np.concatenate([np.asarray(per_core[c][n])
                              for c in range(N_CORES)], axis=0)
              for n in in_names]
    concat += [np.zeros((N_CORES * a.shape[0], *a.shape[1:]), a.dtype)
               for a in out_avals]
    return concat


def _run(per_core):
    sharded, in_names, out_names, out_avals = _get_compiled()
    outs = sharded(*_concat_inputs(per_core, in_names, out_avals))
    res = np.asarray(outs[out_names.index("outT")])
    return res.reshape(N_CORES, D, NT)


def kernel(**inputs):
    np_in = {k: np.asarray(v) for k, v in inputs.items()}
    per_core = _host_inputs(**np_in)
    res = _run(per_core)
    out = np.empty((B, T, D), np.float32)
    for c in range(N_CORES):
        b, s = divmod(c, 4)
        out[b, s * NT:(s + 1) * NT, :] = res[c].T
    return out


def _bench_handles(inputs):
    """Compiled runner + device-resident args, for benchmarking."""
    per_core = _host_inputs(**{k: np.asarray(v) for k, v in inputs.items()})
    sharded, in_names, out_names, out_avals = _get_compiled()
    import jax
    dev_args = [jax.device_put(a)
                for a in _concat_inputs(per_core, in_names, out_avals)]
    return sharded, dev_args
